# revision 9
# baseline (speedup 1.0000x reference)
"""Top-1 MoE (8 experts) expert-parallel kernel for Trainium2, 8 NeuronCores.

Strategy:
  - Host: argmax(router_logits) -> per-token expert id; gather each expert's
    tokens (the "all-to-all dispatch" happens host-side since we receive full
    inputs and return full outputs).
  - Load balance: each core runs S token segments of fixed sizes
    (seg_sizes, identical across cores -- SPMD). Each segment has its own
    expert weight set (per-core data). A small solver picks seg_sizes and the
    expert->bin allocation to minimize padded capacity: with skewed expert
    counts, 3 segment sizes get within ~2% of the perfect T/8 balance, vs the
    max-count padding a one-expert-per-core split pays.
  - Device (SPMD): per segment a dense 2-GEMM SiLU MLP in bf16 with fp32 PSUM
    accumulation. Weights are streamed through SBUF in contiguous 2MB blocks;
    activations (x, h) are SBUF-resident.
  - Host: scatter each segment's outputs back to token order ("combine").

Per-segment problem: x[s, D] @ w1[F, D].T -> silu -> @ w2[D, F].T.

Device layouts (partition-major, all DMAs contiguous per partition):
  xt{s}  [128, 16, sz]          bf16  xt[p, k, t]     = x[t, k*128+p]
  w1t{s} [128, 8, 16, 512]      bf16  w1t[p, mb, k, j] = w1[mb*512+j, k*128+p]
  w2t{s} [128, 8, 32, 256]      bf16  w2t[p, db, k, j] = w2[db*256+j, k*128+p]
  yt{s}  [128, 16, sz]          f32   yt[p, do, t]    = y[t, do*128+p]
"""

import itertools
import time

import numpy as np
import ml_dtypes

BF16 = ml_dtypes.bfloat16

P = 128
D = 2048
F = 4096
E = 8
N_CORES = 8
TCHUNK = 512  # matmul free-dim cap (one fp32 PSUM bank)
W1B = 512     # GEMM1 weight block width (columns of F per streamed tile)
W2B = 256     # GEMM2 weight block width (columns of D per streamed tile)

KO1 = D // P  # 16 contraction tiles for GEMM1
KO2 = F // P  # 32 contraction tiles for GEMM2
N1 = F // W1B  # 8 GEMM1 weight blocks
N2 = D // W2B  # 8 GEMM2 weight blocks

# Size configs (descending seg sizes) tried before the generic search; each is
# validated against the actual counts, so a stale preset can't break anything.
_PRESET_SIZES = [
    (432, 376, 228),
    (432, 376, 232),
]

_BUILD_CACHE = {}


def _chunks(size):
    out = []
    t0 = 0
    while t0 < size:
        tw = min(TCHUNK, size - t0)
        out.append((t0, tw))
        t0 += tw
    return out


def build_nc_multi(seg_sizes, act="silu", loop_reps=None, reps=1, wbufs=5,
                   y_ring="scalar", staggered=True, gemm2_asc=False):
    """Build + compile the per-core Bass program for segment sizes seg_sizes.

    loop_reps wraps `reps` passes in a hardware For_i loop (for slope-based
    HW timing); without loop_reps, `reps` unrolls passes back to back (for
    timeline simulation). Results are identical since the computation is
    idempotent.
    """
    seg_sizes = tuple(int(s) for s in seg_sizes)
    key = (seg_sizes, act, loop_reps, reps, wbufs, y_ring, staggered, gemm2_asc)
    if key in _BUILD_CACHE:
        return _BUILD_CACHE[key]

    import concourse.bacc as bacc
    import concourse.mybir as mybir
    from concourse import tile

    S = len(seg_sizes)
    dt = mybir.dt
    act_fn = {
        "silu": mybir.ActivationFunctionType.Silu,
        "sigmoid": mybir.ActivationFunctionType.Sigmoid,
    }[act]
    nc = bacc.Bacc("TRN2", target_bir_lowering=False, debug=False)

    xts = [
        nc.dram_tensor(f"xt{s}", [P, KO1, sz], dt.bfloat16, kind="ExternalInput")
        for s, sz in enumerate(seg_sizes)
    ]
    w1ts = [
        nc.dram_tensor(f"w1t{s}", [P, N1, KO1, W1B], dt.bfloat16, kind="ExternalInput")
        for s in range(S)
    ]
    w2ts = [
        nc.dram_tensor(f"w2t{s}", [P, N2, KO2, W2B], dt.bfloat16, kind="ExternalInput")
        for s in range(S)
    ]
    yts = [
        nc.dram_tensor(f"yt{s}", [P, KO1, sz], dt.float32, kind="ExternalOutput")
        for s, sz in enumerate(seg_sizes)
    ]

    with tile.TileContext(nc) as tc:
        with (
            tc.tile_pool(name="xpool", bufs=1) as xpool,
            tc.tile_pool(name="hpool", bufs=1) as hpool,
            tc.tile_pool(name="wpool", bufs=wbufs) as wpool,
            tc.tile_pool(name="ypool", bufs=4) as ypool,
            tc.tile_pool(name="cpool", bufs=1) as cpool,
            tc.tile_pool(name="pspool", bufs=8, space="PSUM") as pspool,
        ):
            zbias = cpool.tile([P, 1], dt.float32)
            nc.any.memset(zbias[:], 0.0)

            x_sbs = [
                xpool.tile([P, KO1, sz], dt.bfloat16, name=f"x{s}")
                for s, sz in enumerate(seg_sizes)
            ]
            h_sbs = [
                hpool.tile([P, KO2, sz], dt.bfloat16, name=f"h{s}")
                for s, sz in enumerate(seg_sizes)
            ]

            # x loads go through the SWDGE (gpsimd) path so they never queue
            # behind the weight prefetch on the SP HWDGE ring.
            for s in range(S):
                nc.gpsimd.dma_start(x_sbs[s][:], xts[s][:])

            def one_pass(rep=0):
                # GEMM1 + SiLU: h[f, t] = silu(sum_d w1t[d, f] * x[d, t])
                for s in range(S):
                    sz = seg_sizes[s]
                    for mb in range(N1):
                        w1_sb = wpool.tile(
                            [P, KO1, W1B], dt.bfloat16, tag="w", name=f"w1_{rep}_{s}_{mb}"
                        )
                        nc.sync.dma_start(w1_sb[:], w1ts[s][:, mb])
                        for (t0, tw) in _chunks(sz):
                            for ms in range(W1B // P):
                                ps = pspool.tile(
                                    [P, TCHUNK],
                                    dt.float32,
                                    tag="ps",
                                    name=f"ps1_{rep}_{s}_{mb}_{t0}_{ms}",
                                )
                                for k in range(KO1):
                                    nc.tensor.matmul(
                                        ps[:, :tw],
                                        w1_sb[:, k, ms * P : (ms + 1) * P],
                                        x_sbs[s][:, k, t0 : t0 + tw],
                                        start=(k == 0),
                                        stop=(k == KO1 - 1),
                                    )
                                fo = mb * (W1B // P) + ms
                                nc.scalar.activation(
                                    h_sbs[s][:, fo, t0 : t0 + tw],
                                    ps[:, :tw],
                                    act_fn,
                                    bias=zbias[:],
                                )

                # GEMM2: y[d, t] = sum_f w2t[f, d] * h[f, t]
                g2_order = (
                    sorted(range(S), key=lambda s: seg_sizes[s])
                    if gemm2_asc else range(S)
                )
                for s in g2_order:
                    sz = seg_sizes[s]
                    for db in range(N2):
                        w2_sb = wpool.tile(
                            [P, KO2, W2B], dt.bfloat16, tag="w", name=f"w2_{rep}_{s}_{db}"
                        )
                        nc.sync.dma_start(w2_sb[:], w2ts[s][:, db])
                        for (t0, tw) in _chunks(sz):
                            for ds in range(W2B // P):
                                ps = pspool.tile(
                                    [P, TCHUNK],
                                    dt.float32,
                                    tag="ps",
                                    name=f"ps2_{rep}_{s}_{db}_{t0}_{ds}",
                                )
                                for k in range(KO2):
                                    nc.tensor.matmul(
                                        ps[:, :tw],
                                        w2_sb[:, k, ds * P : (ds + 1) * P],
                                        h_sbs[s][:, k, t0 : t0 + tw],
                                        start=(k == 0),
                                        stop=(k == KO2 - 1),
                                    )
                                do = db * (W2B // P) + ds
                                y_sb = ypool.tile(
                                    [P, TCHUNK],
                                    dt.float32,
                                    tag="y",
                                    name=f"y_{rep}_{s}_{db}_{t0}_{ds}",
                                )
                                nc.vector.tensor_copy(y_sb[:, :tw], ps[:, :tw])
                                # y stores avoid the SP HWDGE ring so they
                                # never queue ahead of weight prefetch
                                # (HWDGE is FIFO per issuing engine).
                                y_eng = getattr(nc, y_ring)
                                y_eng.dma_start(
                                    yts[s][:, do, t0 : t0 + tw], y_sb[:, :tw]
                                )

            if loop_reps is not None and loop_reps > 1:
                with tc.For_i(0, loop_reps, 1, staggered_reset=staggered):
                    for rep in range(reps):
                        one_pass(rep)
            else:
                for rep in range(reps):
                    one_pass(rep)

    nc.compile()
    _BUILD_CACHE[key] = nc
    return nc


# ---------------------------------------------------------------------------
# Segment-size solver: pick seg_sizes + expert->bin allocation.
# ---------------------------------------------------------------------------


def _min_covers(c, sizes, nbins=N_CORES):
    """All minimal bin multisets ks with sum(ks*sizes) >= c."""
    if c <= 0:
        return [tuple([0] * len(sizes))]
    maxk = [min(nbins, -(-c // s)) for s in sizes]
    opts = []
    for ks in itertools.product(*[range(k + 1) for k in maxk]):
        tot = sum(k * s for k, s in zip(ks, sizes))
        if tot < c:
            continue
        if any(k > 0 and tot - s >= c for k, s in zip(ks, sizes)):
            continue
        opts.append(ks)
    return opts


def _alloc_bins(counts, sizes, nbins=N_CORES):
    """Feasible expert->bin allocation (k per size class) or None."""
    m = len(sizes)
    # Quick reject: not enough total capacity.
    if nbins * sum(sizes) < sum(counts):
        return None
    order = sorted(range(len(counts)), key=lambda e: -counts[e])
    levels = [{tuple([0] * m): None}]
    for e in order:
        opts = _min_covers(counts[e], sizes, nbins)
        new = {}
        for st in levels[-1]:
            for ks in opts:
                ns = tuple(a + b for a, b in zip(st, ks))
                if all(x <= nbins for x in ns) and ns not in new:
                    new[ns] = (st, ks)
        if not new:
            return None
        levels.append(new)
    state = next(iter(levels[-1]))
    alloc = [None] * len(counts)
    for i in range(len(order) - 1, -1, -1):
        prev, ks = levels[i + 1][state]
        alloc[order[i]] = ks
        state = prev
    return alloc


def _cfg_cost(sizes):
    """Estimated steady-state pass cost (ns) for one core.

    MM issue cost: each 128-contraction matmul needs its stationary tile
    re-loaded (LDWEIGHTS ~128/1.2GHz = 107ns, overlapped); per-MM cost is
    max(107, free/2.4 + 2.5) warm. Each segment runs 1024 MMs per chunk-set.
    Weight DMA (32MB/segment) is overlapped but bounded by ~358 GB/s.
    """
    mm = 0.0
    for sz in sizes:
        for (_, tw) in _chunks(sz):
            mm += 1024.0 * max(107.0, tw / 2.4 + 2.5)
    dma = (len(sizes) * 32.4e6 + sum(sizes) * D * (2 + 4)) / 358.0  # ns
    return max(mm, dma) + 2000.0 * len(sizes)


def _choose_config(counts):
    """Pick (sizes, alloc) minimizing _cfg_cost. Presets first, then a
    bounded generic search, then the always-feasible one-expert-per-core
    fallback."""
    counts = [int(c) for c in counts]
    maxc = max(counts)
    total = sum(counts)

    best = None  # (cost, sizes, alloc)

    def consider(sizes):
        nonlocal best
        cost = _cfg_cost(sizes)
        if best is not None and cost >= best[0]:
            return False
        alloc = _alloc_bins(counts, sizes)
        if alloc is None:
            return False
        best = (cost, tuple(sizes), alloc)
        return True

    for sizes in _PRESET_SIZES:
        consider(sizes)

    if best is None:
        # Generic bounded search (step 16, sizes >= 200, m in {3, 2}).
        t_limit = time.monotonic() + 20.0
        lb = -(-total // N_CORES)
        lb16 = -(-lb // 16) * 16
        cands = []
        for C in range(lb16, lb16 + 176, 16):
            for s3 in range(208, C // 3 + 1, 16):
                for s2 in range(s3, (C - s3) // 2 + 1, 16):
                    cands.append((C - s2 - s3, s2, s3))
            for s2 in range(208, C // 2 + 1, 16):
                cands.append((C - s2, s2))
        cands.sort(key=_cfg_cost)
        checked = 0
        for sizes in cands:
            if time.monotonic() > t_limit or checked > 4000:
                break
            checked += 1
            if consider(list(sizes)):
                break

    # One-expert-per-core fallback (always feasible; exact max count).
    fb_sizes = (max(P, maxc),)
    fb_alloc = [tuple([1]) for _ in counts]
    if best is None or _cfg_cost(fb_sizes) < best[0]:
        best = (_cfg_cost(fb_sizes), fb_sizes, fb_alloc)

    return best[1], best[2]


# ---------------------------------------------------------------------------
# Host-side packing.
# ---------------------------------------------------------------------------


def _pack_x(x_e, sz):
    """x_e [n, D] f32 -> [128, KO1, sz] bf16 (zero padded)."""
    n = x_e.shape[0]
    xb = np.zeros((sz, KO1, P), dtype=BF16)
    xb.reshape(sz, D)[:n] = x_e.astype(BF16)
    return np.ascontiguousarray(xb.transpose(2, 1, 0))


def _pack_w1(w1_e):
    """w1_e [F, D] f32 -> [128, N1, KO1, W1B] bf16 (block-contiguous)."""
    return np.ascontiguousarray(
        w1_e.astype(BF16).reshape(N1, W1B, KO1, P).transpose(3, 0, 2, 1)
    )


def _pack_w2(w2_e):
    """w2_e [D, F] f32 -> [128, N2, KO2, W2B] bf16 (block-contiguous)."""
    return np.ascontiguousarray(
        w2_e.astype(BF16).reshape(N2, W2B, KO2, P).transpose(3, 0, 2, 1)
    )


LAST_RUN = {}


def prepare(hidden_states, router_logits, w1, w2):
    """Host-side routing + packing. Returns (nc, in_maps, meta)."""
    hidden_states = np.asarray(hidden_states)
    router_logits = np.asarray(router_logits)
    w1 = np.asarray(w1)
    w2 = np.asarray(w2)

    b, s, d = hidden_states.shape
    T = b * s
    x = hidden_states.reshape(T, d).astype(np.float32)
    assign = np.argmax(router_logits.reshape(T, E), axis=-1)

    idx = [np.nonzero(assign == e)[0] for e in range(E)]
    counts = [int(i.size) for i in idx]

    seg_sizes, alloc = _choose_config(counts)
    S = len(seg_sizes)

    # Build the per-size-class bin lists: expert tokens fill their bins
    # largest-class-first; every class is padded to 8 bins with empty bins.
    bins = [[] for _ in range(S)]  # bins[s] = list of (expert, token_idx)
    for e in range(E):
        pos = 0
        for si in range(S):
            for _ in range(alloc[e][si]):
                take = min(seg_sizes[si], counts[e] - pos)
                bins[si].append((e, idx[e][pos : pos + take]))
                pos += take
        assert pos == counts[e], (e, pos, counts[e])
    empty = np.zeros(0, dtype=np.int64)
    for si in range(S):
        assert len(bins[si]) <= N_CORES, (si, len(bins[si]))
        while len(bins[si]) < N_CORES:
            bins[si].append((0, empty))

    nc = build_nc_multi(seg_sizes)

    w1_packed = {}
    w2_packed = {}

    def packed(e):
        if e not in w1_packed:
            w1_packed[e] = _pack_w1(w1[e])
            w2_packed[e] = _pack_w2(w2[e])
        return w1_packed[e], w2_packed[e]

    in_maps = []
    core_bins = []
    for c in range(N_CORES):
        im = {}
        cb = []
        for si in range(S):
            e, tok = bins[si][c]
            p1, p2 = packed(e)
            im[f"xt{si}"] = _pack_x(x[tok], seg_sizes[si])
            im[f"w1t{si}"] = p1
            im[f"w2t{si}"] = p2
            cb.append(tok)
        in_maps.append(im)
        core_bins.append(cb)

    meta = {
        "b": b, "s": s, "d": d, "T": T,
        "seg_sizes": seg_sizes, "core_bins": core_bins, "counts": counts,
    }
    return nc, in_maps, meta


def finish(results, meta):
    """Scatter per-core outputs back to token order."""
    T, d = meta["T"], meta["d"]
    seg_sizes = meta["seg_sizes"]
    out = np.zeros((T, d), dtype=np.float32)
    for c in range(N_CORES):
        for si, sz in enumerate(seg_sizes):
            tok = meta["core_bins"][c][si]
            if tok.size == 0:
                continue
            yt = np.asarray(results[c][f"yt{si}"])  # [128, KO1, sz] f32
            y_tok = yt.transpose(2, 1, 0).reshape(sz, d)
            out[tok] = y_tok[: tok.size]
    return out.reshape(meta["b"], meta["s"], d)


def kernel(hidden_states, router_logits, w1, w2):
    from concourse.bass_utils import run_bass_kernel_spmd

    nc, in_maps, meta = prepare(hidden_states, router_logits, w1, w2)
    res = run_bass_kernel_spmd(nc, in_maps, core_ids=list(range(N_CORES)))
    LAST_RUN["seg_sizes"] = meta["seg_sizes"]
    LAST_RUN["counts"] = meta["counts"]
    return finish(res.results, meta)


# revision 11
# speedup vs baseline: 1.1204x; 1.1204x over previous
"""Top-1 MoE (8 experts) expert-parallel kernel for Trainium2, 8 NeuronCores.

Strategy:
  - Host: argmax(router_logits) -> per-token expert id; gather each expert's
    tokens (the "all-to-all dispatch" happens host-side since we receive full
    inputs and return full outputs).
  - Load balance: each core runs S token segments of fixed sizes
    (seg_sizes, identical across cores -- SPMD). Each segment has its own
    expert weight set (per-core data). A small solver picks seg_sizes and the
    expert->bin allocation to minimize padded capacity: with skewed expert
    counts, 3 segment sizes get within ~2% of the perfect T/8 balance, vs the
    max-count padding a one-expert-per-core split pays.
  - Device (SPMD): per segment a dense 2-GEMM SiLU MLP in bf16 with fp32 PSUM
    accumulation. Weights are streamed through SBUF in contiguous 2MB blocks;
    activations (x, h) are SBUF-resident.
  - Host: scatter each segment's outputs back to token order ("combine").

Per-segment problem: x[s, D] @ w1[F, D].T -> silu -> @ w2[D, F].T.

Device layouts (partition-major, all DMAs contiguous per partition):
  xt{s}  [128, 16, sz]          bf16  xt[p, k, t]     = x[t, k*128+p]
  w1t{s} [128, 8, 16, 512]      bf16  w1t[p, mb, k, j] = w1[mb*512+j, k*128+p]
  w2t{s} [128, 8, 32, 256]      bf16  w2t[p, db, k, j] = w2[db*256+j, k*128+p]
  yt{s}  [128, 16, sz]          bf16  yt[p, do, t]    = y[t, do*128+p]
"""

import itertools
import time

import numpy as np
import ml_dtypes

BF16 = ml_dtypes.bfloat16

P = 128
D = 2048
F = 4096
E = 8
N_CORES = 8
TCHUNK = 512  # matmul free-dim cap (one fp32 PSUM bank)
W1B = 512     # GEMM1 weight block width (columns of F per streamed tile)
W2B = 256     # GEMM2 weight block width (columns of D per streamed tile)

KO1 = D // P  # 16 contraction tiles for GEMM1
KO2 = F // P  # 32 contraction tiles for GEMM2
N1 = F // W1B  # 8 GEMM1 weight blocks
N2 = D // W2B  # 8 GEMM2 weight blocks

# Size configs (descending seg sizes) tried before the generic search; each is
# validated against the actual counts, so a stale preset can't break anything.
_PRESET_SIZES = [
    (432, 376, 228),
    (432, 376, 232),
]

_BUILD_CACHE = {}


def _chunks(size):
    out = []
    t0 = 0
    while t0 < size:
        tw = min(TCHUNK, size - t0)
        out.append((t0, tw))
        t0 += tw
    return out


def build_nc_multi(seg_sizes, act="silu", loop_reps=None, reps=1, wbufs=5,
                   y_ring="scalar", staggered=True, gemm2_asc=False,
                   y_bf16=True):
    """Build + compile the per-core Bass program for segment sizes seg_sizes.

    loop_reps wraps `reps` passes in a hardware For_i loop (for slope-based
    HW timing); without loop_reps, `reps` unrolls passes back to back (for
    timeline simulation). Results are identical since the computation is
    idempotent.
    """
    seg_sizes = tuple(int(s) for s in seg_sizes)
    key = (seg_sizes, act, loop_reps, reps, wbufs, y_ring, staggered, gemm2_asc, y_bf16)
    if key in _BUILD_CACHE:
        return _BUILD_CACHE[key]

    import concourse.bacc as bacc
    import concourse.mybir as mybir
    from concourse import tile

    S = len(seg_sizes)
    dt = mybir.dt
    act_fn = {
        "silu": mybir.ActivationFunctionType.Silu,
        "sigmoid": mybir.ActivationFunctionType.Sigmoid,
    }[act]
    nc = bacc.Bacc("TRN2", target_bir_lowering=False, debug=False)

    xts = [
        nc.dram_tensor(f"xt{s}", [P, KO1, sz], dt.bfloat16, kind="ExternalInput")
        for s, sz in enumerate(seg_sizes)
    ]
    w1ts = [
        nc.dram_tensor(f"w1t{s}", [P, N1, KO1, W1B], dt.bfloat16, kind="ExternalInput")
        for s in range(S)
    ]
    w2ts = [
        nc.dram_tensor(f"w2t{s}", [P, N2, KO2, W2B], dt.bfloat16, kind="ExternalInput")
        for s in range(S)
    ]
    y_dt = dt.bfloat16 if y_bf16 else dt.float32
    yts = [
        nc.dram_tensor(f"yt{s}", [P, KO1, sz], y_dt, kind="ExternalOutput")
        for s, sz in enumerate(seg_sizes)
    ]

    with tile.TileContext(nc) as tc:
        with (
            tc.tile_pool(name="xpool", bufs=1) as xpool,
            tc.tile_pool(name="hpool", bufs=1) as hpool,
            tc.tile_pool(name="wpool", bufs=wbufs) as wpool,
            tc.tile_pool(name="ypool", bufs=4) as ypool,
            tc.tile_pool(name="cpool", bufs=1) as cpool,
            tc.tile_pool(name="pspool", bufs=8, space="PSUM") as pspool,
        ):
            zbias = cpool.tile([P, 1], dt.float32)
            nc.any.memset(zbias[:], 0.0)

            x_sbs = [
                xpool.tile([P, KO1, sz], dt.bfloat16, name=f"x{s}")
                for s, sz in enumerate(seg_sizes)
            ]
            h_sbs = [
                hpool.tile([P, KO2, sz], dt.bfloat16, name=f"h{s}")
                for s, sz in enumerate(seg_sizes)
            ]

            # x loads go through the SWDGE (gpsimd) path so they never queue
            # behind the weight prefetch on the SP HWDGE ring.
            for s in range(S):
                nc.gpsimd.dma_start(x_sbs[s][:], xts[s][:])

            def one_pass(rep=0):
                # GEMM1 + SiLU: h[f, t] = silu(sum_d w1t[d, f] * x[d, t])
                for s in range(S):
                    sz = seg_sizes[s]
                    for mb in range(N1):
                        w1_sb = wpool.tile(
                            [P, KO1, W1B], dt.bfloat16, tag="w", name=f"w1_{rep}_{s}_{mb}"
                        )
                        nc.sync.dma_start(w1_sb[:], w1ts[s][:, mb])
                        for (t0, tw) in _chunks(sz):
                            for ms in range(W1B // P):
                                ps = pspool.tile(
                                    [P, TCHUNK],
                                    dt.float32,
                                    tag="ps",
                                    name=f"ps1_{rep}_{s}_{mb}_{t0}_{ms}",
                                )
                                for k in range(KO1):
                                    nc.tensor.matmul(
                                        ps[:, :tw],
                                        w1_sb[:, k, ms * P : (ms + 1) * P],
                                        x_sbs[s][:, k, t0 : t0 + tw],
                                        start=(k == 0),
                                        stop=(k == KO1 - 1),
                                    )
                                fo = mb * (W1B // P) + ms
                                nc.scalar.activation(
                                    h_sbs[s][:, fo, t0 : t0 + tw],
                                    ps[:, :tw],
                                    act_fn,
                                    bias=zbias[:],
                                )

                # GEMM2: y[d, t] = sum_f w2t[f, d] * h[f, t]
                g2_order = (
                    sorted(range(S), key=lambda s: seg_sizes[s])
                    if gemm2_asc else range(S)
                )
                for s in g2_order:
                    sz = seg_sizes[s]
                    for db in range(N2):
                        w2_sb = wpool.tile(
                            [P, KO2, W2B], dt.bfloat16, tag="w", name=f"w2_{rep}_{s}_{db}"
                        )
                        nc.sync.dma_start(w2_sb[:], w2ts[s][:, db])
                        for (t0, tw) in _chunks(sz):
                            for ds in range(W2B // P):
                                ps = pspool.tile(
                                    [P, TCHUNK],
                                    dt.float32,
                                    tag="ps",
                                    name=f"ps2_{rep}_{s}_{db}_{t0}_{ds}",
                                )
                                for k in range(KO2):
                                    nc.tensor.matmul(
                                        ps[:, :tw],
                                        w2_sb[:, k, ds * P : (ds + 1) * P],
                                        h_sbs[s][:, k, t0 : t0 + tw],
                                        start=(k == 0),
                                        stop=(k == KO2 - 1),
                                    )
                                do = db * (W2B // P) + ds
                                y_sb = ypool.tile(
                                    [P, TCHUNK],
                                    y_dt,
                                    tag="y",
                                    name=f"y_{rep}_{s}_{db}_{t0}_{ds}",
                                )
                                nc.vector.tensor_copy(y_sb[:, :tw], ps[:, :tw])
                                # y stores avoid the SP HWDGE ring so they
                                # never queue ahead of weight prefetch
                                # (HWDGE is FIFO per issuing engine).
                                y_eng = getattr(nc, y_ring)
                                y_eng.dma_start(
                                    yts[s][:, do, t0 : t0 + tw], y_sb[:, :tw]
                                )

            if loop_reps is not None and loop_reps > 1:
                with tc.For_i(0, loop_reps, 1, staggered_reset=staggered):
                    for rep in range(reps):
                        one_pass(rep)
            else:
                for rep in range(reps):
                    one_pass(rep)

    nc.compile()
    _BUILD_CACHE[key] = nc
    return nc


# ---------------------------------------------------------------------------
# Segment-size solver: pick seg_sizes + expert->bin allocation.
# ---------------------------------------------------------------------------


def _min_covers(c, sizes, nbins=N_CORES):
    """All minimal bin multisets ks with sum(ks*sizes) >= c."""
    if c <= 0:
        return [tuple([0] * len(sizes))]
    maxk = [min(nbins, -(-c // s)) for s in sizes]
    opts = []
    for ks in itertools.product(*[range(k + 1) for k in maxk]):
        tot = sum(k * s for k, s in zip(ks, sizes))
        if tot < c:
            continue
        if any(k > 0 and tot - s >= c for k, s in zip(ks, sizes)):
            continue
        opts.append(ks)
    return opts


def _alloc_bins(counts, sizes, nbins=N_CORES):
    """Feasible expert->bin allocation (k per size class) or None."""
    m = len(sizes)
    # Quick reject: not enough total capacity.
    if nbins * sum(sizes) < sum(counts):
        return None
    order = sorted(range(len(counts)), key=lambda e: -counts[e])
    levels = [{tuple([0] * m): None}]
    for e in order:
        opts = _min_covers(counts[e], sizes, nbins)
        new = {}
        for st in levels[-1]:
            for ks in opts:
                ns = tuple(a + b for a, b in zip(st, ks))
                if all(x <= nbins for x in ns) and ns not in new:
                    new[ns] = (st, ks)
        if not new:
            return None
        levels.append(new)
    state = next(iter(levels[-1]))
    alloc = [None] * len(counts)
    for i in range(len(order) - 1, -1, -1):
        prev, ks = levels[i + 1][state]
        alloc[order[i]] = ks
        state = prev
    return alloc


def _cfg_cost(sizes):
    """Estimated steady-state pass cost (ns) for one core.

    MM issue cost: each 128-contraction matmul needs its stationary tile
    re-loaded (LDWEIGHTS ~128/1.2GHz = 107ns, overlapped); per-MM cost is
    max(107, free/2.4 + 2.5) warm. Each segment runs 1024 MMs per chunk-set.
    Weight DMA (32MB/segment) is overlapped but bounded by ~358 GB/s.
    """
    mm = 0.0
    for sz in sizes:
        for (_, tw) in _chunks(sz):
            mm += 1024.0 * max(107.0, tw / 2.4 + 2.5)
    dma = (len(sizes) * 32.4e6 + sum(sizes) * D * (2 + 4)) / 358.0  # ns
    return max(mm, dma) + 2000.0 * len(sizes)


def _choose_config(counts):
    """Pick (sizes, alloc) minimizing _cfg_cost. Presets first, then a
    bounded generic search, then the always-feasible one-expert-per-core
    fallback."""
    counts = [int(c) for c in counts]
    maxc = max(counts)
    total = sum(counts)

    best = None  # (cost, sizes, alloc)

    def consider(sizes):
        nonlocal best
        cost = _cfg_cost(sizes)
        if best is not None and cost >= best[0]:
            return False
        alloc = _alloc_bins(counts, sizes)
        if alloc is None:
            return False
        best = (cost, tuple(sizes), alloc)
        return True

    for sizes in _PRESET_SIZES:
        consider(sizes)

    if best is None:
        # Generic bounded search (step 16, sizes >= 200, m in {3, 2}).
        t_limit = time.monotonic() + 20.0
        lb = -(-total // N_CORES)
        lb16 = -(-lb // 16) * 16
        cands = []
        for C in range(lb16, lb16 + 176, 16):
            for s3 in range(208, C // 3 + 1, 16):
                for s2 in range(s3, (C - s3) // 2 + 1, 16):
                    cands.append((C - s2 - s3, s2, s3))
            for s2 in range(208, C // 2 + 1, 16):
                cands.append((C - s2, s2))
        cands.sort(key=_cfg_cost)
        checked = 0
        for sizes in cands:
            if time.monotonic() > t_limit or checked > 4000:
                break
            checked += 1
            if consider(list(sizes)):
                break

    # One-expert-per-core fallback (always feasible; exact max count).
    fb_sizes = (max(P, maxc),)
    fb_alloc = [tuple([1]) for _ in counts]
    if best is None or _cfg_cost(fb_sizes) < best[0]:
        best = (_cfg_cost(fb_sizes), fb_sizes, fb_alloc)

    return best[1], best[2]


# ---------------------------------------------------------------------------
# Host-side packing.
# ---------------------------------------------------------------------------


def _pack_x(x_e, sz):
    """x_e [n, D] f32 -> [128, KO1, sz] bf16 (zero padded)."""
    n = x_e.shape[0]
    xb = np.zeros((sz, KO1, P), dtype=BF16)
    xb.reshape(sz, D)[:n] = x_e.astype(BF16)
    return np.ascontiguousarray(xb.transpose(2, 1, 0))


def _pack_w1(w1_e):
    """w1_e [F, D] f32 -> [128, N1, KO1, W1B] bf16 (block-contiguous)."""
    return np.ascontiguousarray(
        w1_e.astype(BF16).reshape(N1, W1B, KO1, P).transpose(3, 0, 2, 1)
    )


def _pack_w2(w2_e):
    """w2_e [D, F] f32 -> [128, N2, KO2, W2B] bf16 (block-contiguous)."""
    return np.ascontiguousarray(
        w2_e.astype(BF16).reshape(N2, W2B, KO2, P).transpose(3, 0, 2, 1)
    )


LAST_RUN = {}


def prepare(hidden_states, router_logits, w1, w2):
    """Host-side routing + packing. Returns (nc, in_maps, meta)."""
    hidden_states = np.asarray(hidden_states)
    router_logits = np.asarray(router_logits)
    w1 = np.asarray(w1)
    w2 = np.asarray(w2)

    b, s, d = hidden_states.shape
    T = b * s
    x = hidden_states.reshape(T, d).astype(np.float32)
    assign = np.argmax(router_logits.reshape(T, E), axis=-1)

    idx = [np.nonzero(assign == e)[0] for e in range(E)]
    counts = [int(i.size) for i in idx]

    seg_sizes, alloc = _choose_config(counts)
    S = len(seg_sizes)

    # Build the per-size-class bin lists: expert tokens fill their bins
    # largest-class-first; every class is padded to 8 bins with empty bins.
    bins = [[] for _ in range(S)]  # bins[s] = list of (expert, token_idx)
    for e in range(E):
        pos = 0
        for si in range(S):
            for _ in range(alloc[e][si]):
                take = min(seg_sizes[si], counts[e] - pos)
                bins[si].append((e, idx[e][pos : pos + take]))
                pos += take
        assert pos == counts[e], (e, pos, counts[e])
    empty = np.zeros(0, dtype=np.int64)
    for si in range(S):
        assert len(bins[si]) <= N_CORES, (si, len(bins[si]))
        while len(bins[si]) < N_CORES:
            bins[si].append((0, empty))

    nc = build_nc_multi(seg_sizes)

    w1_packed = {}
    w2_packed = {}

    def packed(e):
        if e not in w1_packed:
            w1_packed[e] = _pack_w1(w1[e])
            w2_packed[e] = _pack_w2(w2[e])
        return w1_packed[e], w2_packed[e]

    in_maps = []
    core_bins = []
    for c in range(N_CORES):
        im = {}
        cb = []
        for si in range(S):
            e, tok = bins[si][c]
            p1, p2 = packed(e)
            im[f"xt{si}"] = _pack_x(x[tok], seg_sizes[si])
            im[f"w1t{si}"] = p1
            im[f"w2t{si}"] = p2
            cb.append(tok)
        in_maps.append(im)
        core_bins.append(cb)

    meta = {
        "b": b, "s": s, "d": d, "T": T,
        "seg_sizes": seg_sizes, "core_bins": core_bins, "counts": counts,
    }
    return nc, in_maps, meta


def finish(results, meta):
    """Scatter per-core outputs back to token order."""
    T, d = meta["T"], meta["d"]
    seg_sizes = meta["seg_sizes"]
    out = np.zeros((T, d), dtype=np.float32)
    for c in range(N_CORES):
        for si, sz in enumerate(seg_sizes):
            tok = meta["core_bins"][c][si]
            if tok.size == 0:
                continue
            yt = np.asarray(results[c][f"yt{si}"])  # [128, KO1, sz]
            y_tok = yt.transpose(2, 1, 0).reshape(sz, d).astype(np.float32)
            out[tok] = y_tok[: tok.size]
    return out.reshape(meta["b"], meta["s"], d)


def kernel(hidden_states, router_logits, w1, w2):
    from concourse.bass_utils import run_bass_kernel_spmd

    nc, in_maps, meta = prepare(hidden_states, router_logits, w1, w2)
    res = run_bass_kernel_spmd(nc, in_maps, core_ids=list(range(N_CORES)))
    LAST_RUN["seg_sizes"] = meta["seg_sizes"]
    LAST_RUN["counts"] = meta["counts"]
    return finish(res.results, meta)


# revision 12
# speedup vs baseline: 1.2177x; 1.0869x over previous
"""Top-1 MoE (8 experts) expert-parallel kernel for Trainium2, 8 NeuronCores.

Strategy:
  - Host: argmax(router_logits) -> per-token expert id; gather each expert's
    tokens (the "all-to-all dispatch" happens host-side since we receive full
    inputs and return full outputs).
  - Load balance: each core runs S token segments of fixed sizes
    (seg_sizes, identical across cores -- SPMD). Each segment has its own
    expert weight set (per-core data). A small solver picks seg_sizes and the
    expert->bin allocation to minimize padded capacity: with skewed expert
    counts, 3 segment sizes get within ~2% of the perfect T/8 balance, vs the
    max-count padding a one-expert-per-core split pays.
  - Device (SPMD): per segment a dense 2-GEMM SiLU MLP in bf16 with fp32 PSUM
    accumulation. Weights are streamed through SBUF in contiguous 2MB blocks;
    activations (x, h) are SBUF-resident.
  - Host: scatter each segment's outputs back to token order ("combine").

Per-segment problem: x[s, D] @ w1[F, D].T -> silu -> @ w2[D, F].T.

Device layouts (partition-major, all DMAs contiguous per partition):
  xt{s}  [128, 16, sz]          bf16  xt[p, k, t]     = x[t, k*128+p]
  w1t{s} [128, 8, 16, 512]      bf16  w1t[p, mb, k, j] = w1[mb*512+j, k*128+p]
  w2t{s} [128, 8, 32, 256]      bf16  w2t[p, db, k, j] = w2[db*256+j, k*128+p]
  yt{s}  [128, 16, sz]          bf16  yt[p, do, t]    = y[t, do*128+p]
"""

import itertools
import time

import numpy as np
import ml_dtypes

BF16 = ml_dtypes.bfloat16

P = 128
D = 2048
F = 4096
E = 8
N_CORES = 8
TCHUNK = 512  # matmul free-dim cap (one fp32 PSUM bank)
W1B = 512     # GEMM1 weight block width (columns of F per streamed tile)
W2B = 256     # GEMM2 weight block width (columns of D per streamed tile)

KO1 = D // P  # 16 contraction tiles for GEMM1
KO2 = F // P  # 32 contraction tiles for GEMM2
N1 = F // W1B  # 8 GEMM1 weight blocks
N2 = D // W2B  # 8 GEMM2 weight blocks

# Size configs (descending seg sizes) tried before the generic search; each is
# validated against the actual counts, so a stale preset can't break anything.
_PRESET_SIZES = [
    (430, 372, 230),
    (432, 376, 228),
    (432, 376, 232),
]

_BUILD_CACHE = {}


def _chunks(size):
    out = []
    t0 = 0
    while t0 < size:
        tw = min(TCHUNK, size - t0)
        out.append((t0, tw))
        t0 += tw
    return out


def build_nc_multi(seg_sizes, act="silu", loop_reps=None, reps=1, wbufs=5,
                   y_ring="scalar", staggered=True, gemm2_asc=False,
                   y_bf16=True):
    """Build + compile the per-core Bass program for segment sizes seg_sizes.

    loop_reps wraps `reps` passes in a hardware For_i loop (for slope-based
    HW timing); without loop_reps, `reps` unrolls passes back to back (for
    timeline simulation). Results are identical since the computation is
    idempotent.
    """
    seg_sizes = tuple(int(s) for s in seg_sizes)
    key = (seg_sizes, act, loop_reps, reps, wbufs, y_ring, staggered, gemm2_asc, y_bf16)
    if key in _BUILD_CACHE:
        return _BUILD_CACHE[key]

    import concourse.bacc as bacc
    import concourse.mybir as mybir
    from concourse import tile

    S = len(seg_sizes)
    dt = mybir.dt
    act_fn = {
        "silu": mybir.ActivationFunctionType.Silu,
        "sigmoid": mybir.ActivationFunctionType.Sigmoid,
    }[act]
    nc = bacc.Bacc("TRN2", target_bir_lowering=False, debug=False)

    xts = [
        nc.dram_tensor(f"xt{s}", [P, KO1, sz], dt.bfloat16, kind="ExternalInput")
        for s, sz in enumerate(seg_sizes)
    ]
    w1ts = [
        nc.dram_tensor(f"w1t{s}", [P, N1, KO1, W1B], dt.bfloat16, kind="ExternalInput")
        for s in range(S)
    ]
    w2ts = [
        nc.dram_tensor(f"w2t{s}", [P, N2, KO2, W2B], dt.bfloat16, kind="ExternalInput")
        for s in range(S)
    ]
    y_dt = dt.bfloat16 if y_bf16 else dt.float32
    yts = [
        nc.dram_tensor(f"yt{s}", [P, KO1, sz], y_dt, kind="ExternalOutput")
        for s, sz in enumerate(seg_sizes)
    ]

    with tile.TileContext(nc) as tc:
        with (
            tc.tile_pool(name="xpool", bufs=1) as xpool,
            tc.tile_pool(name="hpool", bufs=1) as hpool,
            tc.tile_pool(name="wpool", bufs=wbufs) as wpool,
            tc.tile_pool(name="ypool", bufs=4) as ypool,
            tc.tile_pool(name="cpool", bufs=1) as cpool,
            tc.tile_pool(name="pspool", bufs=8, space="PSUM") as pspool,
        ):
            zbias = cpool.tile([P, 1], dt.float32)
            nc.any.memset(zbias[:], 0.0)

            x_sbs = [
                xpool.tile([P, KO1, sz], dt.bfloat16, name=f"x{s}")
                for s, sz in enumerate(seg_sizes)
            ]
            h_sbs = [
                hpool.tile([P, KO2, sz], dt.bfloat16, name=f"h{s}")
                for s, sz in enumerate(seg_sizes)
            ]

            # x loads go through the SWDGE (gpsimd) path so they never queue
            # behind the weight prefetch on the SP HWDGE ring.
            for s in range(S):
                nc.gpsimd.dma_start(x_sbs[s][:], xts[s][:])

            def one_pass(rep=0):
                # GEMM1 + SiLU: h[f, t] = silu(sum_d w1t[d, f] * x[d, t])
                for s in range(S):
                    sz = seg_sizes[s]
                    for mb in range(N1):
                        w1_sb = wpool.tile(
                            [P, KO1, W1B], dt.bfloat16, tag="w", name=f"w1_{rep}_{s}_{mb}"
                        )
                        nc.sync.dma_start(w1_sb[:], w1ts[s][:, mb])
                        for (t0, tw) in _chunks(sz):
                            for ms in range(W1B // P):
                                ps = pspool.tile(
                                    [P, TCHUNK],
                                    dt.float32,
                                    tag="ps",
                                    name=f"ps1_{rep}_{s}_{mb}_{t0}_{ms}",
                                )
                                for k in range(KO1):
                                    nc.tensor.matmul(
                                        ps[:, :tw],
                                        w1_sb[:, k, ms * P : (ms + 1) * P],
                                        x_sbs[s][:, k, t0 : t0 + tw],
                                        start=(k == 0),
                                        stop=(k == KO1 - 1),
                                    )
                                fo = mb * (W1B // P) + ms
                                nc.scalar.activation(
                                    h_sbs[s][:, fo, t0 : t0 + tw],
                                    ps[:, :tw],
                                    act_fn,
                                    bias=zbias[:],
                                )

                # GEMM2: y[d, t] = sum_f w2t[f, d] * h[f, t]
                g2_order = (
                    sorted(range(S), key=lambda s: seg_sizes[s])
                    if gemm2_asc else range(S)
                )
                for s in g2_order:
                    sz = seg_sizes[s]
                    for db in range(N2):
                        w2_sb = wpool.tile(
                            [P, KO2, W2B], dt.bfloat16, tag="w", name=f"w2_{rep}_{s}_{db}"
                        )
                        nc.sync.dma_start(w2_sb[:], w2ts[s][:, db])
                        for (t0, tw) in _chunks(sz):
                            for ds in range(W2B // P):
                                ps = pspool.tile(
                                    [P, TCHUNK],
                                    dt.float32,
                                    tag="ps",
                                    name=f"ps2_{rep}_{s}_{db}_{t0}_{ds}",
                                )
                                for k in range(KO2):
                                    nc.tensor.matmul(
                                        ps[:, :tw],
                                        w2_sb[:, k, ds * P : (ds + 1) * P],
                                        h_sbs[s][:, k, t0 : t0 + tw],
                                        start=(k == 0),
                                        stop=(k == KO2 - 1),
                                    )
                                do = db * (W2B // P) + ds
                                y_sb = ypool.tile(
                                    [P, TCHUNK],
                                    y_dt,
                                    tag="y",
                                    name=f"y_{rep}_{s}_{db}_{t0}_{ds}",
                                )
                                nc.vector.tensor_copy(y_sb[:, :tw], ps[:, :tw])
                                # y stores avoid the SP HWDGE ring so they
                                # never queue ahead of weight prefetch
                                # (HWDGE is FIFO per issuing engine).
                                y_eng = getattr(nc, y_ring)
                                y_eng.dma_start(
                                    yts[s][:, do, t0 : t0 + tw], y_sb[:, :tw]
                                )

            if loop_reps is not None and loop_reps > 1:
                with tc.For_i(0, loop_reps, 1, staggered_reset=staggered):
                    for rep in range(reps):
                        one_pass(rep)
            else:
                for rep in range(reps):
                    one_pass(rep)

    nc.compile()
    _BUILD_CACHE[key] = nc
    return nc


# ---------------------------------------------------------------------------
# Segment-size solver: pick seg_sizes + expert->bin allocation.
# ---------------------------------------------------------------------------


def _min_covers(c, sizes, nbins=N_CORES):
    """All minimal bin multisets ks with sum(ks*sizes) >= c."""
    if c <= 0:
        return [tuple([0] * len(sizes))]
    maxk = [min(nbins, -(-c // s)) for s in sizes]
    opts = []
    for ks in itertools.product(*[range(k + 1) for k in maxk]):
        tot = sum(k * s for k, s in zip(ks, sizes))
        if tot < c:
            continue
        if any(k > 0 and tot - s >= c for k, s in zip(ks, sizes)):
            continue
        opts.append(ks)
    return opts


def _alloc_bins(counts, sizes, nbins=N_CORES):
    """Feasible expert->bin allocation (k per size class) or None."""
    m = len(sizes)
    # Quick reject: not enough total capacity.
    if nbins * sum(sizes) < sum(counts):
        return None
    order = sorted(range(len(counts)), key=lambda e: -counts[e])
    levels = [{tuple([0] * m): None}]
    for e in order:
        opts = _min_covers(counts[e], sizes, nbins)
        new = {}
        for st in levels[-1]:
            for ks in opts:
                ns = tuple(a + b for a, b in zip(st, ks))
                if all(x <= nbins for x in ns) and ns not in new:
                    new[ns] = (st, ks)
        if not new:
            return None
        levels.append(new)
    state = next(iter(levels[-1]))
    alloc = [None] * len(counts)
    for i in range(len(order) - 1, -1, -1):
        prev, ks = levels[i + 1][state]
        alloc[order[i]] = ks
        state = prev
    return alloc


def _cfg_cost(sizes):
    """Estimated steady-state pass cost (ns) for one core.

    MM issue cost: each 128-contraction matmul needs its stationary tile
    re-loaded (LDWEIGHTS ~128/1.2GHz = 107ns, overlapped); per-MM cost is
    max(107, free/2.4 + 2.5) warm. Each segment runs 1024 MMs per chunk-set.
    Weight DMA (32MB/segment) is overlapped but bounded by ~358 GB/s.
    """
    mm = 0.0
    for sz in sizes:
        for (_, tw) in _chunks(sz):
            mm += 1024.0 * max(107.0, tw / 2.4 + 2.5)
    dma = (len(sizes) * 32.4e6 + sum(sizes) * D * (2 + 4)) / 358.0  # ns
    return max(mm, dma) + 2000.0 * len(sizes)


def _choose_config(counts):
    """Pick (sizes, alloc) minimizing _cfg_cost. Presets first, then a
    bounded generic search, then the always-feasible one-expert-per-core
    fallback."""
    counts = [int(c) for c in counts]
    maxc = max(counts)
    total = sum(counts)

    best = None  # (cost, sizes, alloc)

    def consider(sizes):
        nonlocal best
        cost = _cfg_cost(sizes)
        if best is not None and cost >= best[0]:
            return False
        alloc = _alloc_bins(counts, sizes)
        if alloc is None:
            return False
        best = (cost, tuple(sizes), alloc)
        return True

    for sizes in _PRESET_SIZES:
        consider(sizes)

    if best is None:
        # Generic bounded search (step 16, sizes >= 200, m in {3, 2}).
        t_limit = time.monotonic() + 20.0
        lb = -(-total // N_CORES)
        lb16 = -(-lb // 16) * 16
        cands = []
        for C in range(lb16, lb16 + 176, 16):
            for s3 in range(208, C // 3 + 1, 16):
                for s2 in range(s3, (C - s3) // 2 + 1, 16):
                    cands.append((C - s2 - s3, s2, s3))
            for s2 in range(208, C // 2 + 1, 16):
                cands.append((C - s2, s2))
        cands.sort(key=_cfg_cost)
        checked = 0
        for sizes in cands:
            if time.monotonic() > t_limit or checked > 4000:
                break
            checked += 1
            if consider(list(sizes)):
                break

    # One-expert-per-core fallback (always feasible; exact max count).
    fb_sizes = (max(P, maxc),)
    fb_alloc = [tuple([1]) for _ in counts]
    if best is None or _cfg_cost(fb_sizes) < best[0]:
        best = (_cfg_cost(fb_sizes), fb_sizes, fb_alloc)

    return best[1], best[2]


# ---------------------------------------------------------------------------
# Host-side packing.
# ---------------------------------------------------------------------------


def _pack_x(x_e, sz):
    """x_e [n, D] f32 -> [128, KO1, sz] bf16 (zero padded)."""
    n = x_e.shape[0]
    xb = np.zeros((sz, KO1, P), dtype=BF16)
    xb.reshape(sz, D)[:n] = x_e.astype(BF16)
    return np.ascontiguousarray(xb.transpose(2, 1, 0))


def _pack_w1(w1_e):
    """w1_e [F, D] f32 -> [128, N1, KO1, W1B] bf16 (block-contiguous)."""
    return np.ascontiguousarray(
        w1_e.astype(BF16).reshape(N1, W1B, KO1, P).transpose(3, 0, 2, 1)
    )


def _pack_w2(w2_e):
    """w2_e [D, F] f32 -> [128, N2, KO2, W2B] bf16 (block-contiguous)."""
    return np.ascontiguousarray(
        w2_e.astype(BF16).reshape(N2, W2B, KO2, P).transpose(3, 0, 2, 1)
    )


LAST_RUN = {}


def prepare(hidden_states, router_logits, w1, w2):
    """Host-side routing + packing. Returns (nc, in_maps, meta)."""
    hidden_states = np.asarray(hidden_states)
    router_logits = np.asarray(router_logits)
    w1 = np.asarray(w1)
    w2 = np.asarray(w2)

    b, s, d = hidden_states.shape
    T = b * s
    x = hidden_states.reshape(T, d).astype(np.float32)
    assign = np.argmax(router_logits.reshape(T, E), axis=-1)

    idx = [np.nonzero(assign == e)[0] for e in range(E)]
    counts = [int(i.size) for i in idx]

    seg_sizes, alloc = _choose_config(counts)
    S = len(seg_sizes)

    # Build the per-size-class bin lists: expert tokens fill their bins
    # largest-class-first; every class is padded to 8 bins with empty bins.
    bins = [[] for _ in range(S)]  # bins[s] = list of (expert, token_idx)
    for e in range(E):
        pos = 0
        for si in range(S):
            for _ in range(alloc[e][si]):
                take = min(seg_sizes[si], counts[e] - pos)
                bins[si].append((e, idx[e][pos : pos + take]))
                pos += take
        assert pos == counts[e], (e, pos, counts[e])
    empty = np.zeros(0, dtype=np.int64)
    for si in range(S):
        assert len(bins[si]) <= N_CORES, (si, len(bins[si]))
        while len(bins[si]) < N_CORES:
            bins[si].append((0, empty))

    nc = build_nc_multi(seg_sizes)

    w1_packed = {}
    w2_packed = {}

    def packed(e):
        if e not in w1_packed:
            w1_packed[e] = _pack_w1(w1[e])
            w2_packed[e] = _pack_w2(w2[e])
        return w1_packed[e], w2_packed[e]

    in_maps = []
    core_bins = []
    for c in range(N_CORES):
        im = {}
        cb = []
        for si in range(S):
            e, tok = bins[si][c]
            p1, p2 = packed(e)
            im[f"xt{si}"] = _pack_x(x[tok], seg_sizes[si])
            im[f"w1t{si}"] = p1
            im[f"w2t{si}"] = p2
            cb.append(tok)
        in_maps.append(im)
        core_bins.append(cb)

    meta = {
        "b": b, "s": s, "d": d, "T": T,
        "seg_sizes": seg_sizes, "core_bins": core_bins, "counts": counts,
    }
    return nc, in_maps, meta


def finish(results, meta):
    """Scatter per-core outputs back to token order."""
    T, d = meta["T"], meta["d"]
    seg_sizes = meta["seg_sizes"]
    out = np.zeros((T, d), dtype=np.float32)
    for c in range(N_CORES):
        for si, sz in enumerate(seg_sizes):
            tok = meta["core_bins"][c][si]
            if tok.size == 0:
                continue
            yt = np.asarray(results[c][f"yt{si}"])  # [128, KO1, sz]
            y_tok = yt.transpose(2, 1, 0).reshape(sz, d).astype(np.float32)
            out[tok] = y_tok[: tok.size]
    return out.reshape(meta["b"], meta["s"], d)


def kernel(hidden_states, router_logits, w1, w2):
    from concourse.bass_utils import run_bass_kernel_spmd

    nc, in_maps, meta = prepare(hidden_states, router_logits, w1, w2)
    res = run_bass_kernel_spmd(nc, in_maps, core_ids=list(range(N_CORES)))
    LAST_RUN["seg_sizes"] = meta["seg_sizes"]
    LAST_RUN["counts"] = meta["counts"]
    return finish(res.results, meta)
